# revision 2
# baseline (speedup 1.0000x reference)
"""Trainium2 Bass kernel for GQA attention (nn_Attention_12197707121071).

Strategy v2: shard core = (batch, kv-head-group) over 8 NeuronCores.
  - Core c owns batch bi=c//4 and kv-group g=c%4: its 4 query heads
    [4g..4g+3], 1 kv head, and all 2048 rows of its batch. No KV
    replication (vs head-sharding), x DMA halved, AllGathers shrink to
    4-core groups (one per batch).
  - Host pre-transposes x[bi] -> xT (feat, rows), pre-casts to bf16;
    wq/wk columns perm64'd per head so RoPE acts on partition halves;
    score scale folded into wq/wk; RoPE sign folded into sin table.
  - Device pipeline by 512-row chunks: [rc QKV -> attn(rc) -> ship(rc)]
    x4, then all outproj (the in-order PE queue must never sit behind a
    matmul that waits on a collective). rc0 runs kc-major with 6 live
    PSUM accumulators so the startup DMA prefix overlaps compute; later
    chunks stream xb double-buffered and rotate 3 PSUM banks. RoPE is
    fused into PSUM evacuation (rc-boundary evacs routed via the idle
    ACT engine); causal attention computed transposed (S^T = K @ Q^T),
    exp on ACT, software-pipelined 3 blocks deep; softmax denominator =
    DVE running sum of exp blocks + one ones-matmul per chunk, finalized
    one chunk late to hide the accumulate latency.
  - Ships: one (512,512) bf16 AllGather per row chunk over the 4 cores
    of the same batch (2MB-out AGs; bigger is super-linearly slower);
    the LAST chunk ships per head (4 small AGs) as each head finishes,
    so the gathers overlap the attention tail and the final outproj
    piece waits on a 0.5MB AG only. outproj contracts the gathered 16
    head-chunks against this core's 512 wo columns with 4 rotating PSUM
    accumulators; each accumulator is evacuated (DVE/ACT alternating)
    and stored on its own queue the moment it stops, shortening the
    final drain. Output is bf16 (host upcasts to f32).
  - v3: 24 dependency-free warmup matmuls ramp the PE p-state during the
    startup DMA prefix (first real matmul ~8us vs ~15us); startup DMA
    dispatches spread across the sync/scalar/gpsimd queues, rc0-critical
    pieces first, cos/sin/tri/id/ones deferred behind them; the dead
    duplicate xb1 load is gone. NOTE (measured): any NEFF containing a
    collective runs the PE at ~1.95GHz instead of 2.4GHz from ~16us in
    (eager CC init) - see memory/trn2-attention-kernel-findings.md.
"""

import sys
import numpy as np

for _p in (
    "/root/.axon_site",
    "/root/.axon_site/_ro/trn_rl_repo",
    "/root/.axon_site/_ro/pypackages",
    "/opt/trn_rl_repo",
):
    if _p not in sys.path:
        sys.path.append(_p)

import ml_dtypes

BF16 = ml_dtypes.bfloat16

B, S, DIM = 2, 2048, 2048
N_HEADS = 16
N_KV_HEADS = 4
HEAD_DIM = 128
N_CORES = 8
PE_N = 512
HPC = 4                      # q heads per core
QW = HPC * HEAD_DIM          # 512 q cols per core
OW = DIM // N_KV_HEADS       # 512 out cols per core


# --------------------------------------------------------------------------
# device kernel builder
# --------------------------------------------------------------------------

def build_nc(s=S):
    """Build + compile the SPMD Bass graph (identical on all 8 cores)."""
    from contextlib import ExitStack

    from concourse import bacc, mybir
    import concourse.tile as tile

    dt = mybir.dt
    f32, bf16 = dt.float32, dt.bfloat16
    KC = DIM // 128          # contraction chunks (16)
    RC = s // PE_N           # row chunks (4)
    NF = N_HEADS             # gathered head chunks for out-proj (16)

    nc = bacc.Bacc("TRN2", target_bir_lowering=False, debug=False,
                   num_devices=N_CORES)

    d = {}
    d["xT"] = nc.dram_tensor("xT", [DIM, s], bf16, kind="ExternalInput")
    d["wq"] = nc.dram_tensor("wq", [DIM, QW], bf16, kind="ExternalInput")
    d["wk"] = nc.dram_tensor("wk", [DIM, 128], bf16, kind="ExternalInput")
    d["wv"] = nc.dram_tensor("wv", [DIM, 128], bf16, kind="ExternalInput")
    d["wo"] = nc.dram_tensor("wo", [DIM, OW], bf16, kind="ExternalInput")
    d["cosF"] = nc.dram_tensor("cosF", [128, s], bf16, kind="ExternalInput")
    d["sinPM"] = nc.dram_tensor("sinPM", [128, s], bf16, kind="ExternalInput")
    d["tri"] = nc.dram_tensor("tri", [128, 128], bf16, kind="ExternalInput")
    d["onesw"] = nc.dram_tensor("onesw", [128, 128], bf16, kind="ExternalInput")
    d["ident"] = nc.dram_tensor("ident", [128, 128], bf16, kind="ExternalInput")
    d["out"] = nc.dram_tensor("out", [OW, s], bf16, kind="ExternalOutput")

    # one ship per row chunk (2MB-out AGs are the sweet spot; 4MB is
    # super-linearly slower). qc3 ships in two head-pair pieces so the
    # first piece's AllGather overlaps the tail of attention.
    bounce = [nc.dram_tensor(f"bounce{i}", [QW, PE_N], bf16) for i in range(3)]
    gath = [nc.dram_tensor(f"gath{i}", [N_HEADS * 128, PE_N], bf16)
            for i in range(3)]
    bounce3 = [nc.dram_tensor(f"bounce3{p}", [128, PE_N], bf16)
               for p in range(HPC)]
    gath3 = [nc.dram_tensor(f"gath3{p}", [HPC * 128, PE_N], bf16)
             for p in range(HPC)]
    GROUPS = [[0, 1, 2, 3], [4, 5, 6, 7]]

    Exp = mybir.ActivationFunctionType.Exp

    with tile.TileContext(nc) as tc, ExitStack() as ctx:
        cpool = ctx.enter_context(tc.tile_pool(name="consts", bufs=1))
        apool = ctx.enter_context(tc.tile_pool(name="acts", bufs=1))
        xpool = ctx.enter_context(tc.tile_pool(name="xb", bufs=2))
        tpool = ctx.enter_context(tc.tile_pool(name="tmps", bufs=6))
        epool = ctx.enter_context(tc.tile_pool(name="exps", bufs=8))
        rpool = ctx.enter_context(tc.tile_pool(name="recip", bufs=2))
        gtpool = ctx.enter_context(tc.tile_pool(name="gt", bufs=4))
        ospool = ctx.enter_context(tc.tile_pool(name="os", bufs=2))
        espool = ctx.enter_context(tc.tile_pool(name="es", bufs=2))
        mmps = ctx.enter_context(tc.tile_pool(name="mmps", bufs=2, space="PSUM"))
        stps = ctx.enter_context(tc.tile_pool(name="stps", bufs=3, space="PSUM"))
        otps = ctx.enter_context(tc.tile_pool(name="otps", bufs=2, space="PSUM"))
        dnps = ctx.enter_context(tc.tile_pool(name="dnps", bufs=1, space="PSUM"))

        # ---- constant tiles
        wq_sb = cpool.tile([128, KC, QW], bf16, tag="wq")
        wk_sb = cpool.tile([128, KC, 128], bf16, tag="wk")
        wv_sb = cpool.tile([128, KC, 128], bf16, tag="wv")
        wo_sb = cpool.tile([128, KC, OW], bf16, tag="wo")
        cos_sb = cpool.tile([128, s], bf16, tag="cos")
        sin_sb = cpool.tile([128, s], bf16, tag="sin")
        tri_sb = cpool.tile([128, 128], bf16, tag="tri")
        ones_sb = cpool.tile([128, 128], bf16, tag="ones")
        id_sb = cpool.tile([128, 128], bf16, tag="id")

        # ---- persistent activations
        q_sb = apool.tile([128, HPC, s], bf16, tag="q")    # qT per head (rope'd)
        kT_sb = apool.tile([128, s], bf16, tag="k")        # kT (rope'd)
        vT_sb = apool.tile([128, s], bf16, tag="vt")       # vT (pre-transpose)
        vn_sb = apool.tile([128, KC, 128], bf16, tag="vn")  # v natural per key-blk
        ot_sb = apool.tile([128, HPC, s], bf16, tag="ot")  # normalized attn out^T

        # ---- PE warmup: ~24 dependency-free matmuls ramp the tensor engine
        # p-state while the startup DMAs are still dispatching/in flight.
        warm_w = tpool.tile([128, 128], bf16, tag="c1", name="warm_w")
        warm_x = tpool.tile([128, PE_N], bf16, tag="sw", name="warm_x")
        nc.vector.memset(warm_w[:], 0.0)
        nc.vector.memset(warm_x[:], 0.0)
        warm_p = mmps.tile([128, PE_N], f32, tag="mm", name="warm_p")
        for i in range(24):
            nc.tensor.matmul(warm_p[:], warm_w[:], warm_x[:],
                             start=(i == 0), stop=(i == 23))

        # ---- input DMAs (startup-critical order), dispatch spread over the
        # sync/scalar/vector/gpsimd queues so the ~0.7us-per-dispatch
        # sequencer cost doesn't serialize the startup prefix.
        _dq = [nc.sync, nc.scalar, nc.gpsimd]
        _dqi = [0]

        def _q():
            e = _dq[_dqi[0] % len(_dq)]
            _dqi[0] += 1
            return e

        def load_w(dst, src, k0, k1):
            _q().dma_start(
                out=dst[:, k0:k1, :],
                in_=src.ap().rearrange("(kc p) f -> p kc f", p=128)[:, k0:k1, :])

        def load_xb_part(xb, rc, k0, k1, q=None):
            (q or _q()).dma_start(
                out=xb[:, k0:k1, :],
                in_=d["xT"].ap().rearrange("(kc p) w -> p kc w", p=128)
                [:, k0:k1, rc * PE_N:(rc + 1) * PE_N])

        def load_xb(rc):
            xb = xpool.tile([128, KC, PE_N], bf16, tag="xb", name=f"xb{rc}")
            for k0 in range(0, KC, 4):
                load_xb_part(xb, rc, k0, k0 + 4, q=nc.sync)
            return xb

        xbs = {}
        xb0 = xpool.tile([128, KC, PE_N], bf16, tag="xb", name="xb0")
        xbs[0] = xb0
        # first-needed pieces dispatch first, each on its own queue
        load_w(wk_sb, d["wk"], 0, 4)
        load_xb_part(xb0, 0, 0, 2)
        load_xb_part(xb0, 0, 2, 4)
        load_w(wq_sb, d["wq"], 0, 4)
        load_w(wk_sb, d["wk"], 4, 8)
        load_xb_part(xb0, 0, 4, 8)
        load_w(wv_sb, d["wv"], 0, 8)
        load_w(wq_sb, d["wq"], 4, 8)
        load_w(wk_sb, d["wk"], 8, KC)
        load_xb_part(xb0, 0, 8, 12)
        load_w(wv_sb, d["wv"], 8, KC)
        load_w(wq_sb, d["wq"], 8, 12)
        nc.scalar.dma_start(out=tri_sb[:], in_=d["tri"][:, :])
        nc.gpsimd.dma_start(out=id_sb[:], in_=d["ident"][:, :])
        nc.gpsimd.dma_start(out=ones_sb[:], in_=d["onesw"][:, :])
        nc.scalar.dma_start(out=cos_sb[:], in_=d["cosF"][:, :])
        load_xb_part(xb0, 0, 12, KC)
        load_w(wq_sb, d["wq"], 12, KC)
        nc.scalar.dma_start(out=sin_sb[:], in_=d["sinPM"][:, :])
        xbs[1] = load_xb(1)

        def late_consts():
            nc.sync.dma_start(
                out=wo_sb[:],
                in_=d["wo"].ap().rearrange("(kc p) f -> p kc f", p=128))

        def rope_evac(psum, dst, scol, w=PE_N, on_act=False):
            """dst = rope(psum) in bf16 (RoPE sign folded into sinPM).
            on_act moves the cast+swap to the Scalar engine — used for rc0
            where ACT is idle and the serial DVE chain would stall attn(0)."""
            cp = nc.scalar.copy if on_act else nc.vector.tensor_copy
            c1 = tpool.tile([128, PE_N], bf16, tag="c1", name="c1")
            cp(c1[:, :w], psum[:, :w])
            sw = tpool.tile([128, PE_N], bf16, tag="sw", name="sw")
            cp(sw[0:64, :w], c1[64:128, :w])
            cp(sw[64:128, :w], c1[0:64, :w])
            m1 = tpool.tile([128, PE_N], bf16, tag="m1", name="m1")
            nc.vector.tensor_mul(m1[:, :w], c1[:, :w], cos_sb[:, scol:scol + w])
            nc.vector.tensor_mul(sw[:, :w], sw[:, :w], sin_sb[:, scol:scol + w])
            nc.vector.tensor_add(dst, m1[:, :w], sw[:, :w])

        def w_ap_for(mb, kc):
            if mb == 0:
                return wk_sb[:, kc, :]
            if mb == 5:
                return wv_sb[:, kc, :]
            h = mb - 1
            return wq_sb[:, kc, h * 128:(h + 1) * 128]

        def evac_mb(mb, psum, rc, on_act=False):
            cols = rc * PE_N
            vcp = nc.scalar.copy if on_act else nc.vector.tensor_copy
            if mb == 0:
                rope_evac(psum, kT_sb[:, cols:cols + PE_N], cols, on_act=on_act)
            elif mb == 5:
                vcp(vT_sb[:, cols:cols + PE_N], psum[:])
                for j in range(PE_N // 128):
                    kb = rc * (PE_N // 128) + j
                    tt = stps.tile([128, 128], bf16, tag="st", name=f"tt{kb}")
                    nc.tensor.transpose(
                        tt[:], vT_sb[:, kb * 128:(kb + 1) * 128], id_sb[:])
                    nc.vector.tensor_copy(vn_sb[:, kb, :], tt[:])
            else:
                rope_evac(psum, q_sb[:, mb - 1, cols:cols + PE_N], cols,
                          on_act=on_act)

        def qkv_rc0():
            """Row-chunk 0, kc-major: all 6 output blocks progress with each
            arriving x/w chunk so the startup DMA prefix overlaps compute.
            Uses 6 PSUM banks across the pools (attn has not started yet)."""
            xb = xbs.pop(0)
            # pq3 borrows dnps so attn(0)'s first ST gets a free stps slot
            psums = [mmps.tile([128, PE_N], f32, tag="mm", name="p_k"),
                     mmps.tile([128, PE_N], f32, tag="mm", name="p_q0"),
                     stps.tile([128, PE_N], f32, tag="st", name="p_q1"),
                     stps.tile([128, PE_N], f32, tag="st", name="p_q2"),
                     dnps.tile([128, PE_N], f32, tag="dn", name="p_q3"),
                     otps.tile([128, PE_N], f32, tag="ot", name="p_v")]
            for g in range(0, KC, 4):
                last = g + 4 == KC
                # last group: v and k first so their (PE-side) transposes and
                # kT evac clear before attn(0); evac as blocks complete
                order = [5, 0, 1, 2, 3, 4] if last else [0, 1, 2, 3, 4, 5]
                for mb in order:
                    for kc in range(g, g + 4):
                        nc.tensor.matmul(psums[mb][:], w_ap_for(mb, kc),
                                         xb[:, kc, :],
                                         start=(kc == 0), stop=(kc == KC - 1))
                    if last:
                        evac_mb(mb, psums[mb], 0, on_act=True)
            late_consts()

        def qkv_rc(rc):
            """Project row-chunk rc (weights resident): mb triples rotate 3
            PSUM banks."""
            if rc == 0:
                qkv_rc0()
                return
            xb = xbs.pop(rc)
            if rc + 1 < RC:
                xbs[rc + 1] = load_xb(rc + 1)
            # mb: 0 = k, 1..4 = q heads, 5 = v
            for trip in ((0, 1, 2), (3, 4, 5)):
                psums = [mmps.tile([128, PE_N], f32, tag="mm", name=f"t{mb}_{rc}")
                         for mb in trip[:2]]
                psums.append(stps.tile([128, PE_N], f32, tag="st",
                                       name=f"t{trip[2]}_{rc}"))
                for kc in range(KC):
                    for i, mb in enumerate(trip):
                        nc.tensor.matmul(psums[i][:], w_ap_for(mb, kc),
                                         xb[:, kc, :],
                                         start=(kc == 0), stop=(kc == KC - 1))
                for i, mb in enumerate(trip):
                    evac_mb(mb, psums[i], rc)

        def attn_chunk(h, qc, finalize_prev=None, post_fin=None):
            """Emit ST/exp/PV for (h, qc); the softmax denominator is a DVE
            running sum over exp blocks (single ones-matmul at finalize).
            Returns a finalize closure (dn matmul + recip + normalize) the
            caller emits after the NEXT chunk's first STs, hiding the DVE
            accumulate latency behind PE work."""
            nkb = (qc + 1) * (PE_N // 128)
            otp = otps.tile([128, PE_N], f32, tag="ot", name="otp")
            qs = q_sb[:, h, qc * PE_N:(qc + 1) * PE_N]
            exps = {}
            # E doubles as ex(kb=0); all E += ex_kb adds are emitted only
            # after PV(0) (the reader of the original ex0) so Tile's WAR
            # dep ordering keeps PV(0)'s operand intact. Adds commute.
            E = espool.tile([128, PE_N], bf16, tag="E", name=f"E{h}_{qc}")
            pending_adds = []
            state = {"pv0": False}

            def issue_st(kb):
                off = max(0, (kb - 4 * qc) * 128)
                stp = stps.tile([128, PE_N], f32, tag="st", name=f"st{kb}")
                nc.tensor.matmul(
                    stp[:, off:], kT_sb[:, kb * 128:(kb + 1) * 128],
                    qs[:, off:], start=True, stop=True)
                if kb == 0:
                    ex = E
                else:
                    ex = epool.tile([128, PE_N], bf16, tag="ex", name=f"ex{kb}")
                nc.scalar.activation(ex[:, off:], stp[:, off:], Exp)
                if kb >= 4 * qc:
                    nc.vector.tensor_mul(ex[:, off:off + 128],
                                         ex[:, off:off + 128], tri_sb[:])
                exps[kb] = (ex, off)
                if kb > 0:
                    if state["pv0"]:
                        nc.vector.tensor_add(E[:, off:], E[:, off:], ex[:, off:])
                    else:
                        pending_adds.append((ex, off))

            def issue_pv(kb):
                ex, off = exps.pop(kb)
                nc.tensor.matmul(otp[:, off:], vn_sb[:, kb, :], ex[:, off:],
                                 start=(kb == 0), stop=(kb == nkb - 1))
                if kb == 0:
                    state["pv0"] = True
                    for aex, aoff in pending_adds:
                        nc.vector.tensor_add(E[:, aoff:], E[:, aoff:],
                                             aex[:, aoff:])
                    pending_adds.clear()

            DEPTH = 3
            for kb in range(nkb):
                issue_st(kb)
                if kb == 1 and finalize_prev is not None:
                    finalize_prev()
                    if post_fin is not None:
                        post_fin()
                if kb >= DEPTH:
                    issue_pv(kb - DEPTH)
            for kb in range(max(0, nkb - DEPTH), nkb):
                issue_pv(kb)

            def finalize():
                dnp = dnps.tile([128, PE_N], f32, tag="dn", name="dnp")
                nc.tensor.matmul(dnp[:], ones_sb[:], E[:], start=True, stop=True)
                rc_t = rpool.tile([128, PE_N], f32, tag="rc", name="rc_t")
                nc.vector.reciprocal_approx_fast(out=rc_t[:], in_=dnp[:])
                nc.vector.tensor_mul(
                    ot_sb[:, h, qc * PE_N:(qc + 1) * PE_N], otp[:], rc_t[:])

            return finalize

        def ship(qc):
            nc.sync.dma_start(
                out=bounce[qc][:, :],
                in_=ot_sb[:, :, qc * PE_N:(qc + 1) * PE_N])
            nc.gpsimd.collective_compute(
                "AllGather", mybir.AluOpType.bypass,
                replica_groups=GROUPS,
                ins=[bounce[qc].ap().opt()],
                outs=[gath[qc].ap().opt()])

        def ship3(p):
            # single head p of row chunk 3 — fires as soon as head p is
            # normalized, so its AllGather overlaps the attention tail
            nc.sync.dma_start(
                out=bounce3[p][:, :],
                in_=ot_sb[:, p, 3 * PE_N:4 * PE_N])
            nc.gpsimd.collective_compute(
                "AllGather", mybir.AluOpType.bypass,
                replica_groups=GROUPS,
                ins=[bounce3[p].ap().opt()],
                outs=[gath3[p].ap().opt()])

        g_tiles = {}

        def outproj_load(qc):
            # gathered rows: j*512 + p*4 + h  ->  [p, head=(4j+h), w]
            if qc == 3:
                gts = []
                for p in range(HPC):
                    gt = gtpool.tile([128, HPC, PE_N], bf16, tag="g",
                                     name=f"g3_{p}")
                    nc.sync.dma_start(
                        out=gt[:],
                        in_=gath3[p].ap().rearrange(
                            "(j q) w -> q j w", j=HPC, q=128))
                    gts.append(gt)
                g_tiles[qc] = gts
                return
            gts = []
            for fh in range(2):
                gt = gtpool.tile([128, 2, HPC, PE_N], bf16, tag="g",
                                 name=f"g{qc}_{fh}")
                for j2 in range(2):
                    nc.sync.dma_start(
                        out=gt[:, j2, :, :],
                        in_=gath[qc].ap().rearrange(
                            "(j p h) w -> p j h w", j=HPC, p=128, h=HPC)
                        [:, fh * 2 + j2, :, :])
                gts.append(gt)
            g_tiles[qc] = gts

        def outproj_mm(qc):
            # 4 accumulators rotate 4 PSUM banks (2 borrowed from stps)
            gts = g_tiles.pop(qc)
            psums = [mmps.tile([128, PE_N], f32, tag="mm", name=f"op{qc}_{j}")
                     for j in range(2)]
            psums += [stps.tile([128, PE_N], f32, tag="st", name=f"op{qc}_{j}")
                      for j in range(2, 4)]
            if qc == 3:
                # earliest-shipped heads first so matmuls start before the
                # later per-head AllGathers land
                forder = [(j * HPC + p, gts[p][:, j, :])
                          for p in range(HPC) for j in range(HPC)]
            else:
                forder = [(f, gts[f // 8][:, (f % 8) // 4, f % 4, :])
                          for f in range(NF)]
            osb = ospool.tile([128, 4, PE_N], bf16, tag="os", name=f"os{qc}")
            evac_eng = [nc.vector.tensor_copy, nc.scalar.copy,
                        nc.vector.tensor_copy, nc.scalar.copy]
            store_eng = [nc.sync, nc.scalar, nc.gpsimd, nc.scalar]
            for fi, (f, g_ap) in enumerate(forder):
                for oc in range(4):
                    nc.tensor.matmul(
                        psums[oc][:], wo_sb[:, f, oc * 128:(oc + 1) * 128],
                        g_ap, start=(fi == 0), stop=(fi == NF - 1))
                    if fi == NF - 1:
                        # evac each accumulator as soon as it stops (split
                        # across DVE and GpSimd) and store it immediately on
                        # its own queue, so the final drain is per-oc
                        evac_eng[oc](osb[:, oc, :], psums[oc][:])
                        store_eng[oc].dma_start(
                            out=d["out"].ap()
                            .rearrange("(oc p) w -> p oc w", p=128)
                            [:, oc, qc * PE_N:(qc + 1) * PE_N],
                            in_=osb[:, oc, :])

        # ---- pipeline: all attention first (ships overlap via CC queue),
        # outproj last so the in-order PE queue never head-of-line blocks
        # on an AllGather.
        fin = None
        for qc in range(RC - 1):
            qkv_rc(qc)
            for h in range(HPC):
                fin = attn_chunk(h, qc, finalize_prev=fin)
            fin()          # last head must normalize before its ship
            fin = None
            ship(qc)
        qkv_rc(3)
        # each head of qc3 ships as soon as its (deferred) finalize runs
        fin = attn_chunk(0, 3)
        fin = attn_chunk(1, 3, finalize_prev=fin,
                         post_fin=lambda: ship3(0))
        fin = attn_chunk(2, 3, finalize_prev=fin,
                         post_fin=lambda: ship3(1))
        fin = attn_chunk(3, 3, finalize_prev=fin,
                         post_fin=lambda: ship3(2))
        fin()
        ship3(3)
        outproj_load(0)
        outproj_load(1)
        outproj_mm(0)
        outproj_load(2)
        outproj_mm(1)
        outproj_load(3)
        outproj_mm(2)
        outproj_mm(3)

    nc.compile()
    return nc


# --------------------------------------------------------------------------
# host-side input prep / output assembly
# --------------------------------------------------------------------------

def prep_in_maps(x, freqs_cos, freqs_sin, wq, wk, wv, wo, s=S):
    """Shard + preprocess full fp32 inputs into 8 per-core input dicts."""
    x = np.asarray(x, np.float32)
    xTs = [np.ascontiguousarray(x[bi].T).astype(BF16) for bi in range(B)]

    sc = float(HEAD_DIM) ** -0.25
    perm64 = np.concatenate([np.arange(0, 128, 2), np.arange(1, 128, 2)])
    qcols = np.concatenate([h * 128 + perm64 for h in range(N_HEADS)])
    kcols = np.concatenate([g * 128 + perm64 for g in range(N_KV_HEADS)])
    wq_p = (np.asarray(wq, np.float32) * sc)[:, qcols].astype(BF16)
    wk_p = (np.asarray(wk, np.float32) * sc)[:, kcols].astype(BF16)
    wv_p = np.asarray(wv, np.float32).astype(BF16)
    wo_p = np.asarray(wo, np.float32).astype(BF16)

    cosT = np.asarray(freqs_cos, np.float32).T  # (64, s)
    sinT = np.asarray(freqs_sin, np.float32).T
    cosF = np.ascontiguousarray(np.concatenate([cosT, cosT], axis=0)).astype(BF16)
    sinPM = np.ascontiguousarray(np.concatenate([-sinT, sinT], axis=0)).astype(BF16)

    qi = np.arange(128)[None, :]
    ki = np.arange(128)[:, None]
    tri = (qi >= ki).astype(BF16)
    onesw = np.ones((128, 128), BF16)
    ident = np.eye(128, dtype=BF16)

    in_maps = []
    for c in range(N_CORES):
        bi, g = divmod(c, N_KV_HEADS)
        in_maps.append({
            "xT": xTs[bi],
            "wq": np.ascontiguousarray(wq_p[:, g * QW:(g + 1) * QW]),
            "wk": np.ascontiguousarray(wk_p[:, g * 128:(g + 1) * 128]),
            "wv": np.ascontiguousarray(wv_p[:, g * 128:(g + 1) * 128]),
            "wo": np.ascontiguousarray(wo_p[:, g * OW:(g + 1) * OW]),
            "cosF": cosF,
            "sinPM": sinPM,
            "tri": tri,
            "onesw": onesw,
            "ident": ident,
        })
    return in_maps


def assemble_output(results, s=S):
    out = np.empty((B, s, DIM), np.float32)
    for c in range(N_CORES):
        bi, g = divmod(c, N_KV_HEADS)
        out[bi, :, g * OW:(g + 1) * OW] = \
            results[c]["out"].astype(np.float32).T
    return out


_NC_CACHE = {}


def _get_nc(s=S):
    if s not in _NC_CACHE:
        _NC_CACHE[s] = build_nc(s)
    return _NC_CACHE[s]


def run(inputs, trace=False):
    """Run the kernel; returns (output, BassKernelResults)."""
    from concourse import bass_utils
    nc = _get_nc()
    in_maps = prep_in_maps(**inputs)
    res = bass_utils.run_bass_kernel_spmd(
        nc, in_maps, core_ids=list(range(N_CORES)), trace=trace)
    return assemble_output(res.results), res


def kernel(**inputs):
    out, _ = run(inputs)
    return out



# revision 3
# speedup vs baseline: 1.0103x; 1.0103x over previous
"""Trainium2 Bass kernel for GQA attention (nn_Attention_12197707121071).

Strategy v2: shard core = (batch, kv-head-group) over 8 NeuronCores.
  - Core c owns batch bi=c//4 and kv-group g=c%4: its 4 query heads
    [4g..4g+3], 1 kv head, and all 2048 rows of its batch. No KV
    replication (vs head-sharding), x DMA halved, AllGathers shrink to
    4-core groups (one per batch).
  - Host pre-transposes x[bi] -> xT (feat, rows), pre-casts to bf16;
    wq/wk columns perm64'd per head so RoPE acts on partition halves;
    score scale folded into wq/wk; RoPE sign folded into sin table.
  - Device pipeline by 512-row chunks: [rc QKV -> attn(rc) -> ship(rc)]
    x4, then all outproj (the in-order PE queue must never sit behind a
    matmul that waits on a collective). rc0 runs kc-major with 6 live
    PSUM accumulators so the startup DMA prefix overlaps compute; later
    chunks stream xb double-buffered and rotate 3 PSUM banks. RoPE is
    fused into PSUM evacuation (rc-boundary evacs routed via the idle
    ACT engine); causal attention computed transposed (S^T = K @ Q^T),
    exp on ACT, software-pipelined 3 blocks deep; softmax denominator =
    DVE running sum of exp blocks + one ones-matmul per chunk, finalized
    one chunk late to hide the accumulate latency.
  - Ships: one (512,512) bf16 AllGather per row chunk over the 4 cores
    of the same batch (2MB-out AGs; bigger is super-linearly slower);
    the LAST chunk ships per head (4 small AGs) as each head finishes,
    so the gathers overlap the attention tail and the final outproj
    piece waits on a 0.5MB AG only. outproj contracts the gathered 16
    head-chunks against this core's 512 wo columns with 4 rotating PSUM
    accumulators; each accumulator is evacuated (DVE/ACT alternating)
    and stored on its own queue the moment it stops, shortening the
    final drain. Output is bf16 (host upcasts to f32).
  - v3: 24 dependency-free warmup matmuls ramp the PE p-state during the
    startup DMA prefix (first real matmul ~8us vs ~15us); startup DMA
    dispatches spread across the sync/scalar/gpsimd queues, rc0-critical
    pieces first, cos/sin/tri/id/ones deferred behind them; the dead
    duplicate xb1 load is gone. NOTE (measured): any NEFF containing a
    collective runs the PE at ~1.95GHz instead of 2.4GHz from ~16us in
    (eager CC init) - see memory/trn2-attention-kernel-findings.md.
  - v5: each attention head's last 3 PVs are deferred behind the next
    head's first STs (the ACT exp chain lags the PE ~50ns/block, so the
    in-order PE queue otherwise head-of-line blocks on the final exps);
    ship bounce DMAs dispatch from the idle Scalar queue so the
    serialized AllGather chain starts as early as possible (with the
    faster attention it is the tail critical path).
"""

import sys
import numpy as np

for _p in (
    "/root/.axon_site",
    "/root/.axon_site/_ro/trn_rl_repo",
    "/root/.axon_site/_ro/pypackages",
    "/opt/trn_rl_repo",
):
    if _p not in sys.path:
        sys.path.append(_p)

import ml_dtypes

BF16 = ml_dtypes.bfloat16

B, S, DIM = 2, 2048, 2048
N_HEADS = 16
N_KV_HEADS = 4
HEAD_DIM = 128
N_CORES = 8
PE_N = 512
HPC = 4                      # q heads per core
QW = HPC * HEAD_DIM          # 512 q cols per core
OW = DIM // N_KV_HEADS       # 512 out cols per core


# --------------------------------------------------------------------------
# device kernel builder
# --------------------------------------------------------------------------

def build_nc(s=S):
    """Build + compile the SPMD Bass graph (identical on all 8 cores)."""
    from contextlib import ExitStack

    from concourse import bacc, mybir
    import concourse.tile as tile

    dt = mybir.dt
    f32, bf16 = dt.float32, dt.bfloat16
    KC = DIM // 128          # contraction chunks (16)
    RC = s // PE_N           # row chunks (4)
    NF = N_HEADS             # gathered head chunks for out-proj (16)

    nc = bacc.Bacc("TRN2", target_bir_lowering=False, debug=False,
                   num_devices=N_CORES)

    d = {}
    d["xT"] = nc.dram_tensor("xT", [DIM, s], bf16, kind="ExternalInput")
    d["wq"] = nc.dram_tensor("wq", [DIM, QW], bf16, kind="ExternalInput")
    d["wk"] = nc.dram_tensor("wk", [DIM, 128], bf16, kind="ExternalInput")
    d["wv"] = nc.dram_tensor("wv", [DIM, 128], bf16, kind="ExternalInput")
    d["wo"] = nc.dram_tensor("wo", [DIM, OW], bf16, kind="ExternalInput")
    d["cosF"] = nc.dram_tensor("cosF", [128, s], bf16, kind="ExternalInput")
    d["sinPM"] = nc.dram_tensor("sinPM", [128, s], bf16, kind="ExternalInput")
    d["tri"] = nc.dram_tensor("tri", [128, 128], bf16, kind="ExternalInput")
    d["onesw"] = nc.dram_tensor("onesw", [128, 128], bf16, kind="ExternalInput")
    d["ident"] = nc.dram_tensor("ident", [128, 128], bf16, kind="ExternalInput")
    d["out"] = nc.dram_tensor("out", [OW, s], bf16, kind="ExternalOutput")

    # one ship per row chunk (2MB-out AGs are the sweet spot; 4MB is
    # super-linearly slower). qc3 ships in two head-pair pieces so the
    # first piece's AllGather overlaps the tail of attention.
    bounce = [nc.dram_tensor(f"bounce{i}", [QW, PE_N], bf16) for i in range(3)]
    gath = [nc.dram_tensor(f"gath{i}", [N_HEADS * 128, PE_N], bf16)
            for i in range(3)]
    bounce3 = [nc.dram_tensor(f"bounce3{p}", [128, PE_N], bf16)
               for p in range(HPC)]
    gath3 = [nc.dram_tensor(f"gath3{p}", [HPC * 128, PE_N], bf16)
             for p in range(HPC)]
    GROUPS = [[0, 1, 2, 3], [4, 5, 6, 7]]

    Exp = mybir.ActivationFunctionType.Exp

    with tile.TileContext(nc) as tc, ExitStack() as ctx:
        cpool = ctx.enter_context(tc.tile_pool(name="consts", bufs=1))
        apool = ctx.enter_context(tc.tile_pool(name="acts", bufs=1))
        xpool = ctx.enter_context(tc.tile_pool(name="xb", bufs=2))
        tpool = ctx.enter_context(tc.tile_pool(name="tmps", bufs=6))
        epool = ctx.enter_context(tc.tile_pool(name="exps", bufs=8))
        rpool = ctx.enter_context(tc.tile_pool(name="recip", bufs=2))
        gtpool = ctx.enter_context(tc.tile_pool(name="gt", bufs=4))
        ospool = ctx.enter_context(tc.tile_pool(name="os", bufs=2))
        espool = ctx.enter_context(tc.tile_pool(name="es", bufs=2))
        mmps = ctx.enter_context(tc.tile_pool(name="mmps", bufs=2, space="PSUM"))
        stps = ctx.enter_context(tc.tile_pool(name="stps", bufs=3, space="PSUM"))
        otps = ctx.enter_context(tc.tile_pool(name="otps", bufs=2, space="PSUM"))
        dnps = ctx.enter_context(tc.tile_pool(name="dnps", bufs=1, space="PSUM"))

        # ---- constant tiles
        wq_sb = cpool.tile([128, KC, QW], bf16, tag="wq")
        wk_sb = cpool.tile([128, KC, 128], bf16, tag="wk")
        wv_sb = cpool.tile([128, KC, 128], bf16, tag="wv")
        wo_sb = cpool.tile([128, KC, OW], bf16, tag="wo")
        cos_sb = cpool.tile([128, s], bf16, tag="cos")
        sin_sb = cpool.tile([128, s], bf16, tag="sin")
        tri_sb = cpool.tile([128, 128], bf16, tag="tri")
        ones_sb = cpool.tile([128, 128], bf16, tag="ones")
        id_sb = cpool.tile([128, 128], bf16, tag="id")

        # ---- persistent activations
        q_sb = apool.tile([128, HPC, s], bf16, tag="q")    # qT per head (rope'd)
        kT_sb = apool.tile([128, s], bf16, tag="k")        # kT (rope'd)
        vT_sb = apool.tile([128, s], bf16, tag="vt")       # vT (pre-transpose)
        vn_sb = apool.tile([128, KC, 128], bf16, tag="vn")  # v natural per key-blk
        ot_sb = apool.tile([128, HPC, s], bf16, tag="ot")  # normalized attn out^T

        # ---- PE warmup: ~24 dependency-free matmuls ramp the tensor engine
        # p-state while the startup DMAs are still dispatching/in flight.
        warm_w = tpool.tile([128, 128], bf16, tag="c1", name="warm_w")
        warm_x = tpool.tile([128, PE_N], bf16, tag="sw", name="warm_x")
        nc.vector.memset(warm_w[:], 0.0)
        nc.vector.memset(warm_x[:], 0.0)
        warm_p = mmps.tile([128, PE_N], f32, tag="mm", name="warm_p")
        for i in range(24):
            nc.tensor.matmul(warm_p[:], warm_w[:], warm_x[:],
                             start=(i == 0), stop=(i == 23))

        # ---- input DMAs (startup-critical order), dispatch spread over the
        # sync/scalar/vector/gpsimd queues so the ~0.7us-per-dispatch
        # sequencer cost doesn't serialize the startup prefix.
        _dq = [nc.sync, nc.scalar, nc.gpsimd]
        _dqi = [0]

        def _q():
            e = _dq[_dqi[0] % len(_dq)]
            _dqi[0] += 1
            return e

        def load_w(dst, src, k0, k1):
            _q().dma_start(
                out=dst[:, k0:k1, :],
                in_=src.ap().rearrange("(kc p) f -> p kc f", p=128)[:, k0:k1, :])

        def load_xb_part(xb, rc, k0, k1, q=None):
            (q or _q()).dma_start(
                out=xb[:, k0:k1, :],
                in_=d["xT"].ap().rearrange("(kc p) w -> p kc w", p=128)
                [:, k0:k1, rc * PE_N:(rc + 1) * PE_N])

        def load_xb(rc):
            xb = xpool.tile([128, KC, PE_N], bf16, tag="xb", name=f"xb{rc}")
            for k0 in range(0, KC, 4):
                load_xb_part(xb, rc, k0, k0 + 4, q=nc.sync)
            return xb

        xbs = {}
        xb0 = xpool.tile([128, KC, PE_N], bf16, tag="xb", name="xb0")
        xbs[0] = xb0
        # first-needed pieces dispatch first, each on its own queue
        load_w(wk_sb, d["wk"], 0, 4)
        load_xb_part(xb0, 0, 0, 2)
        load_xb_part(xb0, 0, 2, 4)
        load_w(wq_sb, d["wq"], 0, 4)
        load_w(wk_sb, d["wk"], 4, 8)
        load_xb_part(xb0, 0, 4, 8)
        load_w(wv_sb, d["wv"], 0, 8)
        load_w(wq_sb, d["wq"], 4, 8)
        load_w(wk_sb, d["wk"], 8, KC)
        load_xb_part(xb0, 0, 8, 12)
        load_w(wv_sb, d["wv"], 8, KC)
        load_w(wq_sb, d["wq"], 8, 12)
        nc.scalar.dma_start(out=tri_sb[:], in_=d["tri"][:, :])
        nc.gpsimd.dma_start(out=id_sb[:], in_=d["ident"][:, :])
        nc.gpsimd.dma_start(out=ones_sb[:], in_=d["onesw"][:, :])
        nc.scalar.dma_start(out=cos_sb[:], in_=d["cosF"][:, :])
        load_xb_part(xb0, 0, 12, KC)
        load_w(wq_sb, d["wq"], 12, KC)
        nc.scalar.dma_start(out=sin_sb[:], in_=d["sinPM"][:, :])
        xbs[1] = load_xb(1)

        def late_consts():
            nc.sync.dma_start(
                out=wo_sb[:],
                in_=d["wo"].ap().rearrange("(kc p) f -> p kc f", p=128))

        def rope_evac(psum, dst, scol, w=PE_N, on_act=False):
            """dst = rope(psum) in bf16 (RoPE sign folded into sinPM).
            on_act moves the cast+swap to the Scalar engine — used for rc0
            where ACT is idle and the serial DVE chain would stall attn(0)."""
            cp = nc.scalar.copy if on_act else nc.vector.tensor_copy
            c1 = tpool.tile([128, PE_N], bf16, tag="c1", name="c1")
            cp(c1[:, :w], psum[:, :w])
            sw = tpool.tile([128, PE_N], bf16, tag="sw", name="sw")
            cp(sw[0:64, :w], c1[64:128, :w])
            cp(sw[64:128, :w], c1[0:64, :w])
            m1 = tpool.tile([128, PE_N], bf16, tag="m1", name="m1")
            nc.vector.tensor_mul(m1[:, :w], c1[:, :w], cos_sb[:, scol:scol + w])
            nc.vector.tensor_mul(sw[:, :w], sw[:, :w], sin_sb[:, scol:scol + w])
            nc.vector.tensor_add(dst, m1[:, :w], sw[:, :w])

        def w_ap_for(mb, kc):
            if mb == 0:
                return wk_sb[:, kc, :]
            if mb == 5:
                return wv_sb[:, kc, :]
            h = mb - 1
            return wq_sb[:, kc, h * 128:(h + 1) * 128]

        def evac_mb(mb, psum, rc, on_act=False):
            cols = rc * PE_N
            vcp = nc.scalar.copy if on_act else nc.vector.tensor_copy
            if mb == 0:
                rope_evac(psum, kT_sb[:, cols:cols + PE_N], cols, on_act=on_act)
            elif mb == 5:
                vcp(vT_sb[:, cols:cols + PE_N], psum[:])
                for j in range(PE_N // 128):
                    kb = rc * (PE_N // 128) + j
                    tt = stps.tile([128, 128], bf16, tag="st", name=f"tt{kb}")
                    nc.tensor.transpose(
                        tt[:], vT_sb[:, kb * 128:(kb + 1) * 128], id_sb[:])
                    nc.vector.tensor_copy(vn_sb[:, kb, :], tt[:])
            else:
                rope_evac(psum, q_sb[:, mb - 1, cols:cols + PE_N], cols,
                          on_act=on_act)

        def qkv_rc0():
            """Row-chunk 0, kc-major: all 6 output blocks progress with each
            arriving x/w chunk so the startup DMA prefix overlaps compute.
            Uses 6 PSUM banks across the pools (attn has not started yet)."""
            xb = xbs.pop(0)
            # pq3 borrows dnps so attn(0)'s first ST gets a free stps slot
            psums = [mmps.tile([128, PE_N], f32, tag="mm", name="p_k"),
                     mmps.tile([128, PE_N], f32, tag="mm", name="p_q0"),
                     stps.tile([128, PE_N], f32, tag="st", name="p_q1"),
                     stps.tile([128, PE_N], f32, tag="st", name="p_q2"),
                     dnps.tile([128, PE_N], f32, tag="dn", name="p_q3"),
                     otps.tile([128, PE_N], f32, tag="ot", name="p_v")]
            for g in range(0, KC, 4):
                last = g + 4 == KC
                # last group: v and k first so their (PE-side) transposes and
                # kT evac clear before attn(0); evac as blocks complete
                order = [5, 0, 1, 2, 3, 4] if last else [0, 1, 2, 3, 4, 5]
                for mb in order:
                    for kc in range(g, g + 4):
                        nc.tensor.matmul(psums[mb][:], w_ap_for(mb, kc),
                                         xb[:, kc, :],
                                         start=(kc == 0), stop=(kc == KC - 1))
                    if last:
                        evac_mb(mb, psums[mb], 0, on_act=True)
            late_consts()

        def qkv_rc(rc):
            """Project row-chunk rc (weights resident): mb triples rotate 3
            PSUM banks."""
            if rc == 0:
                qkv_rc0()
                return
            xb = xbs.pop(rc)
            if rc + 1 < RC:
                xbs[rc + 1] = load_xb(rc + 1)
            # mb: 0 = k, 1..4 = q heads, 5 = v
            for trip in ((0, 1, 2), (3, 4, 5)):
                psums = [mmps.tile([128, PE_N], f32, tag="mm", name=f"t{mb}_{rc}")
                         for mb in trip[:2]]
                psums.append(stps.tile([128, PE_N], f32, tag="st",
                                       name=f"t{trip[2]}_{rc}"))
                for kc in range(KC):
                    for i, mb in enumerate(trip):
                        nc.tensor.matmul(psums[i][:], w_ap_for(mb, kc),
                                         xb[:, kc, :],
                                         start=(kc == 0), stop=(kc == KC - 1))
                for i, mb in enumerate(trip):
                    evac_mb(mb, psums[i], rc)

        def attn_chunk(h, qc, finalize_prev=None, post_fin=None,
                       drain_prev=None):
            """Emit ST/exp/PV for (h, qc); the softmax denominator is a DVE
            running sum over exp blocks (single ones-matmul at finalize).
            Returns a finalize closure (dn matmul + recip + normalize) the
            caller emits after the NEXT chunk's first STs, hiding the DVE
            accumulate latency behind PE work."""
            nkb = (qc + 1) * (PE_N // 128)
            otp = otps.tile([128, PE_N], f32, tag="ot", name="otp")
            qs = q_sb[:, h, qc * PE_N:(qc + 1) * PE_N]
            exps = {}
            # E doubles as ex(kb=0); all E += ex_kb adds are emitted only
            # after PV(0) (the reader of the original ex0) so Tile's WAR
            # dep ordering keeps PV(0)'s operand intact. Adds commute.
            E = espool.tile([128, PE_N], bf16, tag="E", name=f"E{h}_{qc}")
            pending_adds = []
            state = {"pv0": False}

            def issue_st(kb):
                off = max(0, (kb - 4 * qc) * 128)
                stp = stps.tile([128, PE_N], f32, tag="st", name=f"st{kb}")
                nc.tensor.matmul(
                    stp[:, off:], kT_sb[:, kb * 128:(kb + 1) * 128],
                    qs[:, off:], start=True, stop=True)
                if kb == 0:
                    ex = E
                else:
                    ex = epool.tile([128, PE_N], bf16, tag="ex", name=f"ex{kb}")
                nc.scalar.activation(ex[:, off:], stp[:, off:], Exp)
                if kb >= 4 * qc:
                    nc.vector.tensor_mul(ex[:, off:off + 128],
                                         ex[:, off:off + 128], tri_sb[:])
                exps[kb] = (ex, off)
                if kb > 0:
                    if state["pv0"]:
                        nc.vector.tensor_add(E[:, off:], E[:, off:], ex[:, off:])
                    else:
                        pending_adds.append((ex, off))

            def issue_pv(kb):
                ex, off = exps.pop(kb)
                nc.tensor.matmul(otp[:, off:], vn_sb[:, kb, :], ex[:, off:],
                                 start=(kb == 0), stop=(kb == nkb - 1))
                if kb == 0:
                    state["pv0"] = True
                    for aex, aoff in pending_adds:
                        nc.vector.tensor_add(E[:, aoff:], E[:, aoff:],
                                             aex[:, aoff:])
                    pending_adds.clear()

            DEPTH = 3
            for kb in range(nkb):
                issue_st(kb)
                if kb == 0 and drain_prev is not None:
                    drain_prev()
                if kb == 1 and finalize_prev is not None:
                    finalize_prev()
                    if post_fin is not None:
                        post_fin()
                if kb >= DEPTH:
                    issue_pv(kb - DEPTH)

            def drain():
                # the last DEPTH PVs wait on the head's final exps (ACT lags
                # the PE by ~50ns/block); deferring them behind the next
                # head's first STs fills that stall
                for kb in range(max(0, nkb - DEPTH), nkb):
                    issue_pv(kb)

            def finalize():
                dnp = dnps.tile([128, PE_N], f32, tag="dn", name="dnp")
                nc.tensor.matmul(dnp[:], ones_sb[:], E[:], start=True, stop=True)
                rc_t = rpool.tile([128, PE_N], f32, tag="rc", name="rc_t")
                nc.vector.reciprocal_approx_fast(out=rc_t[:], in_=dnp[:])
                nc.vector.tensor_mul(
                    ot_sb[:, h, qc * PE_N:(qc + 1) * PE_N], otp[:], rc_t[:])

            return finalize, drain

        def ship(qc):
            # scalar queue: idle at ship time, so the bounce dispatch isn't
            # stuck behind the sync queue's xb streaming and the AllGather
            # chain starts as early as possible
            nc.scalar.dma_start(
                out=bounce[qc][:, :],
                in_=ot_sb[:, :, qc * PE_N:(qc + 1) * PE_N])
            nc.gpsimd.collective_compute(
                "AllGather", mybir.AluOpType.bypass,
                replica_groups=GROUPS,
                ins=[bounce[qc].ap().opt()],
                outs=[gath[qc].ap().opt()])

        def ship3(p):
            # single head p of row chunk 3 — fires as soon as head p is
            # normalized, so its AllGather overlaps the attention tail
            nc.scalar.dma_start(
                out=bounce3[p][:, :],
                in_=ot_sb[:, p, 3 * PE_N:4 * PE_N])
            nc.gpsimd.collective_compute(
                "AllGather", mybir.AluOpType.bypass,
                replica_groups=GROUPS,
                ins=[bounce3[p].ap().opt()],
                outs=[gath3[p].ap().opt()])

        g_tiles = {}

        def outproj_load(qc):
            # gathered rows: j*512 + p*4 + h  ->  [p, head=(4j+h), w]
            if qc == 3:
                gts = []
                for p in range(HPC):
                    gt = gtpool.tile([128, HPC, PE_N], bf16, tag="g",
                                     name=f"g3_{p}")
                    nc.sync.dma_start(
                        out=gt[:],
                        in_=gath3[p].ap().rearrange(
                            "(j q) w -> q j w", j=HPC, q=128))
                    gts.append(gt)
                g_tiles[qc] = gts
                return
            gts = []
            for fh in range(2):
                gt = gtpool.tile([128, 2, HPC, PE_N], bf16, tag="g",
                                 name=f"g{qc}_{fh}")
                for j2 in range(2):
                    nc.sync.dma_start(
                        out=gt[:, j2, :, :],
                        in_=gath[qc].ap().rearrange(
                            "(j p h) w -> p j h w", j=HPC, p=128, h=HPC)
                        [:, fh * 2 + j2, :, :])
                gts.append(gt)
            g_tiles[qc] = gts

        def outproj_mm(qc):
            # 4 accumulators rotate 4 PSUM banks (2 borrowed from stps)
            gts = g_tiles.pop(qc)
            psums = [mmps.tile([128, PE_N], f32, tag="mm", name=f"op{qc}_{j}")
                     for j in range(2)]
            psums += [stps.tile([128, PE_N], f32, tag="st", name=f"op{qc}_{j}")
                      for j in range(2, 4)]
            if qc == 3:
                # earliest-shipped heads first so matmuls start before the
                # later per-head AllGathers land
                forder = [(j * HPC + p, gts[p][:, j, :])
                          for p in range(HPC) for j in range(HPC)]
            else:
                forder = [(f, gts[f // 8][:, (f % 8) // 4, f % 4, :])
                          for f in range(NF)]
            osb = ospool.tile([128, 4, PE_N], bf16, tag="os", name=f"os{qc}")
            evac_eng = [nc.vector.tensor_copy, nc.scalar.copy,
                        nc.vector.tensor_copy, nc.scalar.copy]
            store_eng = [nc.sync, nc.scalar, nc.gpsimd, nc.scalar]
            for fi, (f, g_ap) in enumerate(forder):
                for oc in range(4):
                    nc.tensor.matmul(
                        psums[oc][:], wo_sb[:, f, oc * 128:(oc + 1) * 128],
                        g_ap, start=(fi == 0), stop=(fi == NF - 1))
                    if fi == NF - 1:
                        # evac each accumulator as soon as it stops (split
                        # across DVE and GpSimd) and store it immediately on
                        # its own queue, so the final drain is per-oc
                        evac_eng[oc](osb[:, oc, :], psums[oc][:])
                        store_eng[oc].dma_start(
                            out=d["out"].ap()
                            .rearrange("(oc p) w -> p oc w", p=128)
                            [:, oc, qc * PE_N:(qc + 1) * PE_N],
                            in_=osb[:, oc, :])

        # ---- pipeline: all attention first (ships overlap via CC queue),
        # outproj last so the in-order PE queue never head-of-line blocks
        # on an AllGather.
        fin = None
        for qc in range(RC - 1):
            qkv_rc(qc)
            drain = None
            for h in range(HPC):
                fin, drain = attn_chunk(h, qc, finalize_prev=fin,
                                        drain_prev=drain)
            drain()
            fin()          # last head must normalize before its ship
            fin = None
            ship(qc)
        qkv_rc(3)
        # each head of qc3 ships as soon as its (deferred) finalize runs
        fin, drain = attn_chunk(0, 3)
        fin, drain = attn_chunk(1, 3, finalize_prev=fin, drain_prev=drain,
                                post_fin=lambda: ship3(0))
        fin, drain = attn_chunk(2, 3, finalize_prev=fin, drain_prev=drain,
                                post_fin=lambda: ship3(1))
        fin, drain = attn_chunk(3, 3, finalize_prev=fin, drain_prev=drain,
                                post_fin=lambda: ship3(2))
        drain()
        fin()
        ship3(3)
        outproj_load(0)
        outproj_load(1)
        outproj_mm(0)
        outproj_load(2)
        outproj_mm(1)
        outproj_load(3)
        outproj_mm(2)
        outproj_mm(3)

    nc.compile()
    return nc


# --------------------------------------------------------------------------
# host-side input prep / output assembly
# --------------------------------------------------------------------------

def prep_in_maps(x, freqs_cos, freqs_sin, wq, wk, wv, wo, s=S):
    """Shard + preprocess full fp32 inputs into 8 per-core input dicts."""
    x = np.asarray(x, np.float32)
    xTs = [np.ascontiguousarray(x[bi].T).astype(BF16) for bi in range(B)]

    sc = float(HEAD_DIM) ** -0.25
    perm64 = np.concatenate([np.arange(0, 128, 2), np.arange(1, 128, 2)])
    qcols = np.concatenate([h * 128 + perm64 for h in range(N_HEADS)])
    kcols = np.concatenate([g * 128 + perm64 for g in range(N_KV_HEADS)])
    wq_p = (np.asarray(wq, np.float32) * sc)[:, qcols].astype(BF16)
    wk_p = (np.asarray(wk, np.float32) * sc)[:, kcols].astype(BF16)
    wv_p = np.asarray(wv, np.float32).astype(BF16)
    wo_p = np.asarray(wo, np.float32).astype(BF16)

    cosT = np.asarray(freqs_cos, np.float32).T  # (64, s)
    sinT = np.asarray(freqs_sin, np.float32).T
    cosF = np.ascontiguousarray(np.concatenate([cosT, cosT], axis=0)).astype(BF16)
    sinPM = np.ascontiguousarray(np.concatenate([-sinT, sinT], axis=0)).astype(BF16)

    qi = np.arange(128)[None, :]
    ki = np.arange(128)[:, None]
    tri = (qi >= ki).astype(BF16)
    onesw = np.ones((128, 128), BF16)
    ident = np.eye(128, dtype=BF16)

    in_maps = []
    for c in range(N_CORES):
        bi, g = divmod(c, N_KV_HEADS)
        in_maps.append({
            "xT": xTs[bi],
            "wq": np.ascontiguousarray(wq_p[:, g * QW:(g + 1) * QW]),
            "wk": np.ascontiguousarray(wk_p[:, g * 128:(g + 1) * 128]),
            "wv": np.ascontiguousarray(wv_p[:, g * 128:(g + 1) * 128]),
            "wo": np.ascontiguousarray(wo_p[:, g * OW:(g + 1) * OW]),
            "cosF": cosF,
            "sinPM": sinPM,
            "tri": tri,
            "onesw": onesw,
            "ident": ident,
        })
    return in_maps


def assemble_output(results, s=S):
    out = np.empty((B, s, DIM), np.float32)
    for c in range(N_CORES):
        bi, g = divmod(c, N_KV_HEADS)
        out[bi, :, g * OW:(g + 1) * OW] = \
            results[c]["out"].astype(np.float32).T
    return out


_NC_CACHE = {}


def _get_nc(s=S):
    if s not in _NC_CACHE:
        _NC_CACHE[s] = build_nc(s)
    return _NC_CACHE[s]


def run(inputs, trace=False):
    """Run the kernel; returns (output, BassKernelResults)."""
    from concourse import bass_utils
    nc = _get_nc()
    in_maps = prep_in_maps(**inputs)
    res = bass_utils.run_bass_kernel_spmd(
        nc, in_maps, core_ids=list(range(N_CORES)), trace=trace)
    return assemble_output(res.results), res


def kernel(**inputs):
    out, _ = run(inputs)
    return out

